# revision 17
# baseline (speedup 1.0000x reference)
"""Trainium2 Bass kernel for 16-head attention (B=2, S=2048, D=1024, H=16, K=64).

Sharding: tensor-parallel over heads — each of the 8 cores computes 2 heads'
full attention and the per-head output projection; the head-summed partial
outputs [B, S, D] are reduced on the host (the module sums head outputs, so
the cross-core reduction is a plain numpy sum of 8 partials).

Per-core layout strategy (all matmuls bf16, fp32 PSUM accumulation):
  - Host pre-transposes x to x^T [B, D, S] (bf16) so Q^T/K^T/V come straight
    out of matmuls without on-device transposes.
  - Q^T, K^T computed as [k2=128(2 heads), S] tiles; scores are computed
    *transposed* (scores^T[t, s] tiles) so the attention x V matmul needs no
    transpose of the S x S score matrix.
  - V is augmented with a ones column ([V_h | 1]) so the same matmul that
    accumulates ctx^T also produces the softmax denominators (row 64).
  - exp runs on ScalarE from PSUM in [128, 1024] tiles (scale=1/8 folded into
    the activation's free affine); no max-subtraction (scores/8 are in ±2.5).
  - Softmax normalization: DVE reciprocal of the denominator row, DMA
    partition-broadcast of the reciprocal, one DVE tensor_tensor multiply.
"""

import sys
import numpy as np

for _p in ("/opt/trn_rl_repo",):
    if _p not in sys.path:
        sys.path.insert(0, _p)

import ml_dtypes

B, S, D, H, K = 2, 2048, 1024, 16, 64
NCORES = 8
HPC = H // NCORES          # heads per core
K2 = HPC * K               # 128: stacked per-core head dim
SC = 512                   # s-chunk in the attention inner loop
NSC = S // SC              # 4
NT = S // 128              # 16 key/value tiles
NDT = D // 128             # 8 contraction tiles for projections

_BF16 = ml_dtypes.bfloat16
_COMPILED = None


def _build():
    from contextlib import ExitStack
    import concourse.bass as bass
    import concourse.tile as tile
    import concourse.mybir as mybir
    from concourse import bacc

    fp32 = mybir.dt.float32
    bf16 = mybir.dt.bfloat16
    AF = mybir.ActivationFunctionType

    nc = bacc.Bacc("TRN2", target_bir_lowering=False, debug=False,
                   num_devices=NCORES)

    xT = nc.dram_tensor("xt", [B, D, S], bf16, kind="ExternalInput").ap()
    wq = nc.dram_tensor("wq", [D, K2], bf16, kind="ExternalInput").ap()
    wk = nc.dram_tensor("wk", [D, K2], bf16, kind="ExternalInput").ap()
    wv = nc.dram_tensor("wv", [D, K2], bf16, kind="ExternalInput").ap()
    wo = nc.dram_tensor("wo", [K2, D], bf16, kind="ExternalInput").ap()
    out = nc.dram_tensor("out", [B, S, D], fp32, kind="ExternalOutput").ap()

    xT_t = xT.rearrange("b (t p) s -> b t p s", p=128)   # [B, 8, 128, S]
    wq_t = wq.rearrange("(t p) k -> t p k", p=128)       # [8, 128, 128]
    wk_t = wk.rearrange("(t p) k -> t p k", p=128)
    wv_t = wv.rearrange("(t p) k -> t p k", p=128)

    with tile.TileContext(nc) as tc, ExitStack() as ctx:
        consts = ctx.enter_context(tc.tile_pool(name="consts", bufs=1))
        xpool = ctx.enter_context(tc.tile_pool(name="x", bufs=1))
        qkpool = ctx.enter_context(tc.tile_pool(name="qk", bufs=2))
        vpool = ctx.enter_context(tc.tile_pool(name="v", bufs=2))
        epool = ctx.enter_context(tc.tile_pool(name="e", bufs=10))
        cpool = ctx.enter_context(tc.tile_pool(name="ctx", bufs=2))
        npool = ctx.enter_context(tc.tile_pool(name="norm", bufs=2))
        opool = ctx.enter_context(tc.tile_pool(name="osb", bufs=3))
        ps_sc = ctx.enter_context(tc.tile_pool(name="ps_sc", bufs=2, space="PSUM"))
        ps_ctx = ctx.enter_context(tc.tile_pool(name="ps_ctx", bufs=1, space="PSUM"))
        ps_mm = ctx.enter_context(tc.tile_pool(name="ps_mm", bufs=2, space="PSUM"))

        # Weights, resident for the whole kernel.
        wq_sb = consts.tile([128, NDT, K2], bf16)
        wk_sb = consts.tile([128, NDT, K2], bf16)
        wv_sb = consts.tile([128, NDT, K2], bf16)
        wo_sb = consts.tile([128, D], bf16)
        ones_sb = consts.tile([1, K], fp32)
        nc.vector.memset(ones_sb[:], 1.0)
        nc.sync.dma_start(out=wk_sb[:], in_=wk_t.rearrange("t p k -> p t k"))
        nc.gpsimd.dma_start(out=wq_sb[:], in_=wq_t.rearrange("t p k -> p t k"))
        nc.gpsimd.dma_start(out=wv_sb[:], in_=wv_t.rearrange("t p k -> p t k"))

        # x^T resident (both batches): [128, B, 8, S] bf16 = 64 KiB/partition.
        # Loaded in (b, s-chunk, d-tile) order so the first projections can
        # start after ~1 MB instead of after the full 8.4 MB.
        x_sb = xpool.tile([128, B, NDT, S], bf16)
        for sc in range(NSC):
            for dt in range(NDT):
                eng = (nc.sync, nc.gpsimd, nc.scalar)[dt % 3]
                eng.dma_start(
                    out=x_sb[:, 0, dt, sc * SC:(sc + 1) * SC],
                    in_=xT_t[0, dt][:, sc * SC:(sc + 1) * SC])
        nc.gpsimd.dma_start(out=wo_sb[:], in_=wo[:])
        for dt in range(NDT):
            eng = nc.sync if dt % 2 == 0 else nc.gpsimd
            eng.dma_start(out=x_sb[:, 1, dt, :], in_=xT_t[1, dt])

        def emit_oproj_part(b, sc, ctxT_sb, si):
            osb = opool.tile([128, D], fp32, name="osb")
            for dc in range(D // 512):
                ops = ps_mm.tile([128, 512], fp32, tag="mm", name="ps_o")
                nc.tensor.matmul(
                    ops[:],
                    lhsT=ctxT_sb[:, si * 128:(si + 1) * 128],
                    rhs=wo_sb[:, dc * 512:(dc + 1) * 512],
                    start=True, stop=True,
                )
                nc.vector.tensor_copy(osb[:, dc * 512:(dc + 1) * 512], ops[:])
            row0 = sc * SC + si * 128
            nc.sync.dma_start(out=out[b, row0:row0 + 128, :], in_=osb[:])

        def emit_qk_chunk(dst_sb, w_sb, b, c):
            ps = ps_mm.tile([128, SC], fp32, tag="mm", name="ps_qk")
            for dt in range(NDT):
                nc.tensor.matmul(
                    ps[:],
                    lhsT=w_sb[:, dt, :],
                    rhs=x_sb[:, b, dt, c * SC:(c + 1) * SC],
                    start=(dt == 0), stop=(dt == NDT - 1),
                )
            nc.vector.tensor_copy(dst_sb[:, c * SC:(c + 1) * SC], ps[:])

        def emit_v_group(v_sb, b, tg):
            ps = ps_mm.tile([128, 512], fp32, tag="mm", name="ps_v")
            for ti in range(4):
                t = tg * 4 + ti
                for dt in range(NDT):
                    nc.tensor.matmul(
                        ps[:, ti * 128:(ti + 1) * 128],
                        lhsT=x_sb[:, b, dt, t * 128:(t + 1) * 128],
                        rhs=wv_sb[:, dt, :],
                        start=(dt == 0), stop=(dt == NDT - 1),
                    )
            nc.vector.tensor_copy(
                v_sb[:, tg * 4:(tg + 1) * 4, :, 0:64],
                ps.rearrange("p (ti h c) -> p ti h c", ti=4, h=2),
            )

        def emit_normalize2(b, sc, ctx_ps, ctxT_sb, recips):
            # broadcast recip rows across partitions via a K=1 ones matmul,
            # then one multiply per head (PSUM src x SBUF src)
            bps = ps_mm.tile([128, SC], fp32, tag="mm", name="ps_bcast")
            for h in range(HPC):
                nc.tensor.matmul(
                    bps[h * K:(h + 1) * K, :],
                    lhsT=ones_sb[:], rhs=recips[h][:],
                    start=True, stop=True,
                )
            bcast = npool.tile([128, SC], fp32, tag="bcast",
                               name=f"bcast_{b}_{sc}")
            nc.vector.tensor_copy(bcast[:], bps[:])
            for h in range(HPC):
                nc.vector.tensor_tensor(
                    out=ctxT_sb[h * K:(h + 1) * K, :],
                    in0=ctx_ps[h][0:64, :],
                    in1=bcast[h * K:(h + 1) * K, :],
                    op=mybir.AluOpType.mult,
                )

        def emit_normalize(b, sc, ctx_ps, ctxT_sb):
            # denominator rows -> partition 0 (custom DVE ops require start
            # partition 0), fast reciprocal, DMA partition-broadcast, multiply
            recips = []
            for h in range(HPC):
                den = npool.tile([1, SC], fp32, tag=f"den{h}",
                                 name=f"den{h}_{b}_{sc}")
                nc.scalar.copy(den[:], ctx_ps[h][64:65, :])
                recip = npool.tile([1, SC], fp32, tag=f"recip{h}",
                                   name=f"recip{h}_{b}_{sc}")
                nc.vector.reciprocal_approx_fast(recip[:], den[:])
                recips.append(recip)
            return recips

        # ---- global software pipeline over g = (b, sc, t) ----
        # At each step g: side-work queue item, scores+exp(g), ctx(g-LAG).
        LAG = 6
        NG = B * NSC * NT  # 128 steps
        side = {}          # g -> list of zero-arg closures

        def at(g, fn, *args):
            side.setdefault(g, []).append((fn, args))

        kt_sb = {}; qt_sb = {}; v_sb = {}
        for b in range(B):
            kt_sb[b] = qkpool.tile([128, S], bf16, tag="kt", name=f"kt_{b}")
            qt_sb[b] = qkpool.tile([128, S], bf16, tag="qt", name=f"qt_{b}")
            v_sb[b] = vpool.tile([128, NT, 2, 65], bf16, name=f"v_{b}")

        # PE warmup: dummy K=1 matmuls during the head DMA wait get the HAM
        # clock-gate to 2.4 GHz before the first real matmul arrives.
        warm_sb = consts.tile([1, SC], fp32)
        nc.vector.memset(warm_sb[:], 1.0)
        warm_ps = ps_sc.tile([128, 2 * SC], fp32, tag="sc_ps", name="warm_ps")
        for w in range(12):
            nc.tensor.matmul(warm_ps[0:1, 0:SC], lhsT=warm_sb[:, 0:1],
                             rhs=warm_sb[:], start=True, stop=True)

        # b0 head: first chunks emitted before the stream starts; the rest are
        # paced into early-b0 steps (kt chunk c is consumed from g=4c).
        emit_qk_chunk(kt_sb[0], wk_sb, 0, 0)
        emit_qk_chunk(qt_sb[0], wq_sb, 0, 0)
        nc.vector.memset(v_sb[0][:, :, :, 64:65], 1.0)
        at(1, emit_qk_chunk, kt_sb[0], wk_sb, 0, 1)
        at(2, emit_v_group, v_sb[0], 0, 0)
        at(3, emit_v_group, v_sb[0], 0, 1)
        at(4, emit_qk_chunk, kt_sb[0], wk_sb, 0, 2)
        at(5, emit_v_group, v_sb[0], 0, 2)
        at(6, emit_qk_chunk, kt_sb[0], wk_sb, 0, 3)
        at(8, emit_v_group, v_sb[0], 0, 3)
        at(9, emit_qk_chunk, qt_sb[0], wq_sb, 0, 1)
        at(11, emit_qk_chunk, qt_sb[0], wq_sb, 0, 2)
        at(13, emit_qk_chunk, qt_sb[0], wq_sb, 0, 3)

        # b1 projections: spread through b0's attention (x[b1] is loaded by
        # then; one group roughly every 4 steps keeps PE ahead of ACT).
        def memset_v1():
            nc.vector.memset(v_sb[1][:, :, :, 64:65], 1.0)
        at(18, memset_v1)
        at(20, emit_qk_chunk, kt_sb[1], wk_sb, 1, 0)
        at(24, emit_qk_chunk, kt_sb[1], wk_sb, 1, 1)
        at(28, emit_qk_chunk, kt_sb[1], wk_sb, 1, 2)
        at(32, emit_qk_chunk, kt_sb[1], wk_sb, 1, 3)
        at(36, emit_qk_chunk, qt_sb[1], wq_sb, 1, 0)
        at(40, emit_v_group, v_sb[1], 1, 0)
        at(44, emit_v_group, v_sb[1], 1, 1)
        at(48, emit_v_group, v_sb[1], 1, 2)
        at(52, emit_v_group, v_sb[1], 1, 3)
        at(56, emit_qk_chunk, qt_sb[1], wq_sb, 1, 1)
        at(58, emit_qk_chunk, qt_sb[1], wq_sb, 1, 2)
        at(60, emit_qk_chunk, qt_sb[1], wq_sb, 1, 3)

        ctx_ps_by_sc = {}
        ctxT_by_sc = {}
        e_tiles = {}

        def decode(g):
            b = g // (NSC * NT)
            sc = (g // NT) % NSC
            t = g % NT
            return b, sc, t

        for g in range(NG + LAG):
            for fn, args in side.get(g, ()):
                fn(*args)
            if g < NG:
                b, sc, t = decode(g)
                sc_ps = ps_sc.tile([128, 2 * SC], fp32)
                for h in range(HPC):
                    nc.tensor.matmul(
                        sc_ps[:, h * SC:(h + 1) * SC],
                        lhsT=kt_sb[b][h * K:(h + 1) * K, t * 128:(t + 1) * 128],
                        rhs=qt_sb[b][h * K:(h + 1) * K, sc * SC:(sc + 1) * SC],
                        start=True, stop=True,
                        tile_position=(h * K, 0),
                    )
                e_sb = epool.tile([128, 2 * SC], bf16)
                nc.scalar.activation(e_sb[:], sc_ps[:], AF.Exp,
                                     scale=float(1.0 / np.sqrt(K)))
                e_tiles[g] = e_sb
            gc = g - LAG
            if gc >= 0:
                b, sc, t = decode(gc)
                if t == 0:
                    ctx_ps_by_sc[(b, sc)] = [
                        ps_ctx.tile([65, SC], fp32, tag=f"ctx{h}",
                                    name=f"ctx_ps{h}_{b}_{sc}")
                        for h in range(HPC)]
                e_sb = e_tiles.pop(gc)
                ctx_ps = ctx_ps_by_sc[(b, sc)]
                for h in range(HPC):
                    nc.tensor.matmul(
                        ctx_ps[h][:],
                        lhsT=v_sb[b][:, t, h, :],
                        rhs=e_sb[:, h * SC:(h + 1) * SC],
                        start=(t == 0), stop=(t == NT - 1),
                    )
                if t == NT - 1:
                    ctxT_sb = cpool.tile([128, SC], bf16,
                                         name=f"ctxT_{b}_{sc}")
                    ctxT_by_sc[(b, sc)] = ctxT_sb
                    recips = emit_normalize(b, sc, ctx_ps, ctxT_sb)
                    at(g + 2, emit_normalize2, b, sc, ctx_ps, ctxT_sb, recips)
                    # spread the output projection over the next steps
                    for si in range(SC // 128):
                        at(g + 4 + 2 * si, emit_oproj_part, b, sc, ctxT_sb, si)

        # drain any side work scheduled past the end of the stream
        for g in sorted(k for k in side if k >= NG + LAG):
            for fn, args in side[g]:
                fn(*args)

    nc.compile()
    return nc


def _get_compiled():
    global _COMPILED
    if _COMPILED is None:
        _COMPILED = _build()
    return _COMPILED


def _make_in_maps(x, Wq, Wk, Wv, Wo):
    xT = np.ascontiguousarray(x.transpose(0, 2, 1)).astype(_BF16)  # [B, D, S]
    in_maps = []
    for c in range(NCORES):
        h0 = c * HPC
        wq_c = np.concatenate([Wq[h0 + i] for i in range(HPC)], axis=1)
        wk_c = np.concatenate([Wk[h0 + i] for i in range(HPC)], axis=1)
        wv_c = np.concatenate([Wv[h0 + i] for i in range(HPC)], axis=1)
        wo_c = np.concatenate([Wo[h0 + i] for i in range(HPC)], axis=0)
        in_maps.append({
            "xt": xT,
            "wq": np.ascontiguousarray(wq_c).astype(_BF16),
            "wk": np.ascontiguousarray(wk_c).astype(_BF16),
            "wv": np.ascontiguousarray(wv_c).astype(_BF16),
            "wo": np.ascontiguousarray(wo_c).astype(_BF16),
        })
    return in_maps


def run_on_device(x, Wq, Wk, Wv, Wo, trace=False, trace_kwargs=None):
    """Run the SPMD kernel; returns (out, BassKernelResults)."""
    from concourse.bass_utils import run_bass_kernel_spmd
    nc = _get_compiled()
    in_maps = _make_in_maps(np.asarray(x, np.float32), np.asarray(Wq, np.float32),
                            np.asarray(Wk, np.float32), np.asarray(Wv, np.float32),
                            np.asarray(Wo, np.float32))
    res = run_bass_kernel_spmd(nc, in_maps, list(range(NCORES)), trace=trace,
                               **(trace_kwargs or {}))
    acc = np.zeros((B, S, D), np.float32)
    for r in res.results:
        acc += r["out"]
    return acc, res


def kernel(x, Wq, Wk, Wv, Wo):
    out, _ = run_on_device(x, Wq, Wk, Wv, Wo, trace=False)
    return out


# revision 18
# speedup vs baseline: 1.0395x; 1.0395x over previous
"""Trainium2 Bass kernel for 16-head attention (B=2, S=2048, D=1024, H=16, K=64).

Sharding: tensor-parallel over heads — each of the 8 cores computes 2 heads'
full attention and the per-head output projection; the head-summed partial
outputs [B, S, D] are reduced on the host (the module sums head outputs, so
the cross-core reduction is a plain numpy sum of 8 partials).

Per-core layout strategy (all matmuls bf16, fp32 PSUM accumulation):
  - Host pre-transposes x to x^T [B, D, S] (bf16) so Q^T/K^T/V come straight
    out of matmuls without on-device transposes.
  - Q^T, K^T computed as [k2=128(2 heads), S] tiles; scores are computed
    *transposed* (scores^T[t, s] tiles) so the attention x V matmul needs no
    transpose of the S x S score matrix.
  - V is augmented with a ones column ([V_h | 1]) so the same matmul that
    accumulates ctx^T also produces the softmax denominators (row 64).
  - exp runs on ScalarE from PSUM in [128, 1024] tiles (scale=1/8 folded into
    the activation's free affine); no max-subtraction (scores/8 are in ±2.5).
  - Softmax normalization: DVE reciprocal of the denominator row, DMA
    partition-broadcast of the reciprocal, one DVE tensor_tensor multiply.
"""

import sys
import numpy as np

for _p in ("/opt/trn_rl_repo",):
    if _p not in sys.path:
        sys.path.insert(0, _p)

import ml_dtypes

B, S, D, H, K = 2, 2048, 1024, 16, 64
NCORES = 8
HPC = H // NCORES          # heads per core
K2 = HPC * K               # 128: stacked per-core head dim
SC = 512                   # s-chunk in the attention inner loop
NSC = S // SC              # 4
NT = S // 128              # 16 key/value tiles
NDT = D // 128             # 8 contraction tiles for projections

_BF16 = ml_dtypes.bfloat16
_COMPILED = None


def _build():
    from contextlib import ExitStack
    import concourse.bass as bass
    import concourse.tile as tile
    import concourse.mybir as mybir
    from concourse import bacc

    fp32 = mybir.dt.float32
    bf16 = mybir.dt.bfloat16
    AF = mybir.ActivationFunctionType

    nc = bacc.Bacc("TRN2", target_bir_lowering=False, debug=False,
                   num_devices=NCORES)

    xT = nc.dram_tensor("xt", [B, D, S], bf16, kind="ExternalInput").ap()
    wq = nc.dram_tensor("wq", [D, K2], bf16, kind="ExternalInput").ap()
    wk = nc.dram_tensor("wk", [D, K2], bf16, kind="ExternalInput").ap()
    wv = nc.dram_tensor("wv", [D, K2], bf16, kind="ExternalInput").ap()
    wo = nc.dram_tensor("wo", [K2, D], bf16, kind="ExternalInput").ap()
    out = nc.dram_tensor("out", [B, S, D], fp32, kind="ExternalOutput").ap()

    xT_t = xT.rearrange("b (t p) s -> b t p s", p=128)   # [B, 8, 128, S]
    wq_t = wq.rearrange("(t p) k -> t p k", p=128)       # [8, 128, 128]
    wk_t = wk.rearrange("(t p) k -> t p k", p=128)
    wv_t = wv.rearrange("(t p) k -> t p k", p=128)

    with tile.TileContext(nc) as tc, ExitStack() as ctx:
        consts = ctx.enter_context(tc.tile_pool(name="consts", bufs=1))
        xpool = ctx.enter_context(tc.tile_pool(name="x", bufs=1))
        qkpool = ctx.enter_context(tc.tile_pool(name="qk", bufs=2))
        vpool = ctx.enter_context(tc.tile_pool(name="v", bufs=2))
        epool = ctx.enter_context(tc.tile_pool(name="e", bufs=10))
        cpool = ctx.enter_context(tc.tile_pool(name="ctx", bufs=2))
        npool = ctx.enter_context(tc.tile_pool(name="norm", bufs=2))
        opool = ctx.enter_context(tc.tile_pool(name="osb", bufs=3))
        ps_sc = ctx.enter_context(tc.tile_pool(name="ps_sc", bufs=2, space="PSUM"))
        ps_ctx = ctx.enter_context(tc.tile_pool(name="ps_ctx", bufs=1, space="PSUM"))
        ps_mm = ctx.enter_context(tc.tile_pool(name="ps_mm", bufs=2, space="PSUM"))

        # Weights, resident for the whole kernel.
        wq_sb = consts.tile([128, NDT, K2], bf16)
        wk_sb = consts.tile([128, NDT, K2], bf16)
        wv_sb = consts.tile([128, NDT, K2], bf16)
        wo_sb = consts.tile([128, D], bf16)
        ones_sb = consts.tile([1, K], fp32)
        nc.vector.memset(ones_sb[:], 1.0)
        nc.sync.dma_start(out=wk_sb[:], in_=wk_t.rearrange("t p k -> p t k"))
        nc.gpsimd.dma_start(out=wq_sb[:], in_=wq_t.rearrange("t p k -> p t k"))
        nc.gpsimd.dma_start(out=wv_sb[:], in_=wv_t.rearrange("t p k -> p t k"))

        # x^T resident (both batches): [128, B, 8, S] bf16 = 64 KiB/partition.
        # Loaded in (b, s-chunk, d-tile) order so the first projections can
        # start after ~1 MB instead of after the full 8.4 MB.
        x_sb = xpool.tile([128, B, NDT, S], bf16)
        for sc in range(NSC):
            for dt in range(NDT):
                eng = (nc.sync, nc.gpsimd, nc.scalar)[dt % 3]
                eng.dma_start(
                    out=x_sb[:, 0, dt, sc * SC:(sc + 1) * SC],
                    in_=xT_t[0, dt][:, sc * SC:(sc + 1) * SC])
        nc.gpsimd.dma_start(out=wo_sb[:], in_=wo[:])
        for dt in range(NDT):
            eng = nc.sync if dt % 2 == 0 else nc.gpsimd
            eng.dma_start(out=x_sb[:, 1, dt, :], in_=xT_t[1, dt])

        def emit_oproj_part(b, sc, ctxT_sb, si):
            osb = opool.tile([128, D], fp32, name="osb")
            for dc in range(D // 512):
                ops = ps_mm.tile([128, 512], fp32, tag="mm", name="ps_o")
                nc.tensor.matmul(
                    ops[:],
                    lhsT=ctxT_sb[:, si * 128:(si + 1) * 128],
                    rhs=wo_sb[:, dc * 512:(dc + 1) * 512],
                    start=True, stop=True,
                )
                nc.vector.tensor_copy(osb[:, dc * 512:(dc + 1) * 512], ops[:])
            row0 = sc * SC + si * 128
            nc.sync.dma_start(out=out[b, row0:row0 + 128, :], in_=osb[:])

        def emit_qk_chunk(dst_sb, w_sb, b, c):
            ps = ps_mm.tile([128, SC], fp32, tag="mm", name="ps_qk")
            for dt in range(NDT):
                nc.tensor.matmul(
                    ps[:],
                    lhsT=w_sb[:, dt, :],
                    rhs=x_sb[:, b, dt, c * SC:(c + 1) * SC],
                    start=(dt == 0), stop=(dt == NDT - 1),
                )
            nc.vector.tensor_copy(dst_sb[:, c * SC:(c + 1) * SC], ps[:])

        def emit_v_group(v_sb, b, tg):
            ps = ps_mm.tile([128, 512], fp32, tag="mm", name="ps_v")
            for ti in range(4):
                t = tg * 4 + ti
                for dt in range(NDT):
                    nc.tensor.matmul(
                        ps[:, ti * 128:(ti + 1) * 128],
                        lhsT=x_sb[:, b, dt, t * 128:(t + 1) * 128],
                        rhs=wv_sb[:, dt, :],
                        start=(dt == 0), stop=(dt == NDT - 1),
                    )
            nc.vector.tensor_copy(
                v_sb[:, tg * 4:(tg + 1) * 4, :, 0:64],
                ps.rearrange("p (ti h c) -> p ti h c", ti=4, h=2),
            )

        def emit_normalize2(b, sc, ctx_ps, ctxT_sb, recips):
            # broadcast recip rows across partitions via a K=1 ones matmul,
            # then one multiply per head (PSUM src x SBUF src)
            bps = ps_mm.tile([128, SC], fp32, tag="mm", name="ps_bcast")
            for h in range(HPC):
                nc.tensor.matmul(
                    bps[h * K:(h + 1) * K, :],
                    lhsT=ones_sb[:], rhs=recips[h][:],
                    start=True, stop=True,
                )
            bcast = npool.tile([128, SC], fp32, tag="bcast",
                               name=f"bcast_{b}_{sc}")
            nc.vector.tensor_copy(bcast[:], bps[:])
            for h in range(HPC):
                nc.vector.tensor_tensor(
                    out=ctxT_sb[h * K:(h + 1) * K, :],
                    in0=ctx_ps[h][0:64, :],
                    in1=bcast[h * K:(h + 1) * K, :],
                    op=mybir.AluOpType.mult,
                )

        def emit_normalize(b, sc, ctx_ps, ctxT_sb):
            # denominator rows -> partition 0 (custom DVE ops require start
            # partition 0), fast reciprocal, DMA partition-broadcast, multiply
            recips = []
            for h in range(HPC):
                den = npool.tile([1, SC], fp32, tag=f"den{h}",
                                 name=f"den{h}_{b}_{sc}")
                nc.vector.tensor_copy(den[:], ctx_ps[h][64:65, :])
                recip = npool.tile([1, SC], fp32, tag=f"recip{h}",
                                   name=f"recip{h}_{b}_{sc}")
                nc.vector.reciprocal_approx_fast(recip[:], den[:])
                recips.append(recip)
            return recips

        # ---- global software pipeline over g = (b, sc, t) ----
        # At each step g: side-work queue item, scores+exp(g), ctx(g-LAG).
        LAG = 6
        NG = B * NSC * NT  # 128 steps
        side = {}          # g -> list of zero-arg closures

        def at(g, fn, *args):
            side.setdefault(g, []).append((fn, args))

        kt_sb = {}; qt_sb = {}; v_sb = {}
        for b in range(B):
            kt_sb[b] = qkpool.tile([128, S], bf16, tag="kt", name=f"kt_{b}")
            qt_sb[b] = qkpool.tile([128, S], bf16, tag="qt", name=f"qt_{b}")
            v_sb[b] = vpool.tile([128, NT, 2, 65], bf16, name=f"v_{b}")

        # PE warmup: dummy K=1 matmuls during the head DMA wait get the HAM
        # clock-gate to 2.4 GHz before the first real matmul arrives.
        warm_sb = consts.tile([1, SC], fp32)
        nc.vector.memset(warm_sb[:], 1.0)
        warm_ps = ps_sc.tile([128, 2 * SC], fp32, tag="sc_ps", name="warm_ps")
        for w in range(12):
            nc.tensor.matmul(warm_ps[0:1, 0:SC], lhsT=warm_sb[:, 0:1],
                             rhs=warm_sb[:], start=True, stop=True)

        # b0 head: first chunks emitted before the stream starts; the rest are
        # paced into early-b0 steps (kt chunk c is consumed from g=4c).
        emit_qk_chunk(kt_sb[0], wk_sb, 0, 0)
        emit_qk_chunk(qt_sb[0], wq_sb, 0, 0)
        nc.vector.memset(v_sb[0][:, :, :, 64:65], 1.0)
        at(1, emit_qk_chunk, kt_sb[0], wk_sb, 0, 1)
        at(2, emit_v_group, v_sb[0], 0, 0)
        at(3, emit_v_group, v_sb[0], 0, 1)
        at(4, emit_qk_chunk, kt_sb[0], wk_sb, 0, 2)
        at(5, emit_v_group, v_sb[0], 0, 2)
        at(6, emit_qk_chunk, kt_sb[0], wk_sb, 0, 3)
        at(8, emit_v_group, v_sb[0], 0, 3)
        at(9, emit_qk_chunk, qt_sb[0], wq_sb, 0, 1)
        at(11, emit_qk_chunk, qt_sb[0], wq_sb, 0, 2)
        at(13, emit_qk_chunk, qt_sb[0], wq_sb, 0, 3)

        # b1 projections: spread through b0's attention (x[b1] is loaded by
        # then; one group roughly every 4 steps keeps PE ahead of ACT).
        def memset_v1():
            nc.vector.memset(v_sb[1][:, :, :, 64:65], 1.0)
        at(18, memset_v1)
        at(20, emit_qk_chunk, kt_sb[1], wk_sb, 1, 0)
        at(24, emit_qk_chunk, kt_sb[1], wk_sb, 1, 1)
        at(28, emit_qk_chunk, kt_sb[1], wk_sb, 1, 2)
        at(32, emit_qk_chunk, kt_sb[1], wk_sb, 1, 3)
        at(36, emit_qk_chunk, qt_sb[1], wq_sb, 1, 0)
        at(40, emit_v_group, v_sb[1], 1, 0)
        at(44, emit_v_group, v_sb[1], 1, 1)
        at(48, emit_v_group, v_sb[1], 1, 2)
        at(52, emit_v_group, v_sb[1], 1, 3)
        at(56, emit_qk_chunk, qt_sb[1], wq_sb, 1, 1)
        at(58, emit_qk_chunk, qt_sb[1], wq_sb, 1, 2)
        at(60, emit_qk_chunk, qt_sb[1], wq_sb, 1, 3)

        ctx_ps_by_sc = {}
        ctxT_by_sc = {}
        e_tiles = {}

        def decode(g):
            b = g // (NSC * NT)
            sc = (g // NT) % NSC
            t = g % NT
            return b, sc, t

        for g in range(NG + LAG):
            for fn, args in side.get(g, ()):
                fn(*args)
            if g < NG:
                b, sc, t = decode(g)
                sc_ps = ps_sc.tile([128, 2 * SC], fp32)
                for h in range(HPC):
                    nc.tensor.matmul(
                        sc_ps[:, h * SC:(h + 1) * SC],
                        lhsT=kt_sb[b][h * K:(h + 1) * K, t * 128:(t + 1) * 128],
                        rhs=qt_sb[b][h * K:(h + 1) * K, sc * SC:(sc + 1) * SC],
                        start=True, stop=True,
                        tile_position=(h * K, 0),
                    )
                e_sb = epool.tile([128, 2 * SC], bf16)
                nc.scalar.activation(e_sb[:], sc_ps[:], AF.Exp,
                                     scale=float(1.0 / np.sqrt(K)))
                e_tiles[g] = e_sb
            gc = g - LAG
            if gc >= 0:
                b, sc, t = decode(gc)
                if t == 0:
                    ctx_ps_by_sc[(b, sc)] = [
                        ps_ctx.tile([65, SC], fp32, tag=f"ctx{h}",
                                    name=f"ctx_ps{h}_{b}_{sc}")
                        for h in range(HPC)]
                e_sb = e_tiles.pop(gc)
                ctx_ps = ctx_ps_by_sc[(b, sc)]
                for h in range(HPC):
                    nc.tensor.matmul(
                        ctx_ps[h][:],
                        lhsT=v_sb[b][:, t, h, :],
                        rhs=e_sb[:, h * SC:(h + 1) * SC],
                        start=(t == 0), stop=(t == NT - 1),
                    )
                if t == NT - 1:
                    ctxT_sb = cpool.tile([128, SC], bf16,
                                         name=f"ctxT_{b}_{sc}")
                    ctxT_by_sc[(b, sc)] = ctxT_sb
                    recips = emit_normalize(b, sc, ctx_ps, ctxT_sb)
                    at(g + 2, emit_normalize2, b, sc, ctx_ps, ctxT_sb, recips)
                    # spread the output projection over the next steps
                    for si in range(SC // 128):
                        at(g + 4 + 2 * si, emit_oproj_part, b, sc, ctxT_sb, si)

        # drain any side work scheduled past the end of the stream
        for g in sorted(k for k in side if k >= NG + LAG):
            for fn, args in side[g]:
                fn(*args)

    nc.compile()
    return nc


def _get_compiled():
    global _COMPILED
    if _COMPILED is None:
        _COMPILED = _build()
    return _COMPILED


def _make_in_maps(x, Wq, Wk, Wv, Wo):
    xT = np.ascontiguousarray(x.transpose(0, 2, 1)).astype(_BF16)  # [B, D, S]
    in_maps = []
    for c in range(NCORES):
        h0 = c * HPC
        wq_c = np.concatenate([Wq[h0 + i] for i in range(HPC)], axis=1)
        wk_c = np.concatenate([Wk[h0 + i] for i in range(HPC)], axis=1)
        wv_c = np.concatenate([Wv[h0 + i] for i in range(HPC)], axis=1)
        wo_c = np.concatenate([Wo[h0 + i] for i in range(HPC)], axis=0)
        in_maps.append({
            "xt": xT,
            "wq": np.ascontiguousarray(wq_c).astype(_BF16),
            "wk": np.ascontiguousarray(wk_c).astype(_BF16),
            "wv": np.ascontiguousarray(wv_c).astype(_BF16),
            "wo": np.ascontiguousarray(wo_c).astype(_BF16),
        })
    return in_maps


def run_on_device(x, Wq, Wk, Wv, Wo, trace=False, trace_kwargs=None):
    """Run the SPMD kernel; returns (out, BassKernelResults)."""
    from concourse.bass_utils import run_bass_kernel_spmd
    nc = _get_compiled()
    in_maps = _make_in_maps(np.asarray(x, np.float32), np.asarray(Wq, np.float32),
                            np.asarray(Wk, np.float32), np.asarray(Wv, np.float32),
                            np.asarray(Wo, np.float32))
    res = run_bass_kernel_spmd(nc, in_maps, list(range(NCORES)), trace=trace,
                               **(trace_kwargs or {}))
    acc = np.zeros((B, S, D), np.float32)
    for r in res.results:
        acc += r["out"]
    return acc, res


def kernel(x, Wq, Wk, Wv, Wo):
    out, _ = run_on_device(x, Wq, Wk, Wv, Wo, trace=False)
    return out
